# revision 2
# baseline (speedup 1.0000x reference)
"""Multi-head self-attention 2D Bass kernel for Trainium2 — v4.

Problem: x [4, 512, 48, 48] fp32; qkv_w [1536, 512]; proj_w [512, 512].
  qkv 1x1-conv -> per-head attention (8 heads, head_dim 64) over N=2304
  positions -> output projection.

Sharding (8 cores): core i handles batch b = i//2 and query half i%2
  (NQ = 1152 queries). Each core computes K/V for the full image and the
  projection for its query columns; outputs are disjoint slices -> host
  gather, no collectives.

Pipeline design (per core): ScalarE (exp, ~171 us) and TensorE
(~186 us) are the near-saturated engines; everything else is arranged
so neither ever waits:
  - input DMAs split into exactly the ranges the first compute blocks
    need (k weights for pair 0 arrive ~0.4 us in); first exp at ~6 us.
  - only the first k/q chunk and v0..v3 run before the attention loop;
    the rest of pair-0's k/q/v and all later pairs' k/q are emitted as
    deadline-ordered fillers inside the attention m-loops, so the PE
    queue digests them in the ~90 ns/m it has spare while ScalarE owns
    the critical path.
  - per-m software pipelining: AV matmuls for key-tile m-1 are emitted
    after the scores+exp of m, keeping next-score matmuls ahead of AV
    in the PE queue.
  - all PSUM evacuations on VectorE (ScalarE does exp only).
  - softmax denominators land in rows {0,32} of a [33, NQ] tile (1-row
    DVE accesses must start 32-aligned); one reciprocal + one [33,128]
    block-matrix broadcast matmul + one tensor_mul per (pair, chunk).
  - output projection accumulates over head pairs in PSUM per chunk as
    soon as the last pair normalizes that chunk; pair 3 processes the
    128-query remainder first so only a 512-chunk pipeline drains at
    the end; output DMA per (row-tile, chunk).
"""

import numpy as np

B = 4
C = 512
HH = 48
WW = 48
N = HH * WW          # 2304
NQ = N // 2          # 1152 queries per core
HEADS = 8
D = C // HEADS       # 64
SCALE = float(D) ** -0.5
NCORES = 8

CT = C // 128        # 4 channel tiles
MT = N // 128        # 18 key tiles
QCH = [(0, 512), (512, 512), (1024, 128)]           # query chunks
KCH = [(0, 512), (512, 512), (1024, 512), (1536, 512), (2048, 256)]

_CACHE: dict = {}


def _build_module():
    import concourse.mybir as mybir
    import concourse.tile as tile
    from concourse import bacc

    FP16 = mybir.dt.float16
    FP32 = mybir.dt.float32
    AF = mybir.ActivationFunctionType

    nc = bacc.Bacc("TRN2", target_bir_lowering=False, debug=False)
    xk = nc.dram_tensor("xk", [C, N], FP16, kind="ExternalInput")
    xq = nc.dram_tensor("xq", [C, NQ], FP16, kind="ExternalInput")
    wqkv = nc.dram_tensor("wqkv", [C, 3 * C], FP16, kind="ExternalInput")
    wproj = nc.dram_tensor("wproj", [C, C], FP16, kind="ExternalInput")
    y = nc.dram_tensor("y", [C, NQ], FP32, kind="ExternalOutput")

    with tile.TileContext(nc) as tc:
        with (
            tc.tile_pool(name="consts", bufs=1) as cpool,
            tc.tile_pool(name="wts", bufs=1) as wpool,
            tc.tile_pool(name="qkv", bufs=1) as qkpool,
            tc.tile_pool(name="keep", bufs=1) as keep,
            tc.tile_pool(name="esb", bufs=10) as epool,
            tc.tile_pool(name="ps1", bufs=2, space="PSUM") as ps1,
            tc.tile_pool(name="sps", bufs=2, space="PSUM") as spool,
            tc.tile_pool(name="avps", bufs=1, space="PSUM") as avp,
        ):
            # block-broadcast matrix: out rows 0:64 <- rhs row0, 64:128 <- row32
            m2 = cpool.tile([33, 128], FP16, name="m2", tag="m2")
            nc.vector.memset(m2[0:32, :], 0.0)
            nc.vector.memset(m2[32:33, :], 0.0)
            nc.vector.memset(m2[0:1, 0:64], 1.0)
            nc.vector.memset(m2[32:33, 64:128], 1.0)

            wt = [wpool.tile([128, 3 * C], FP16, name=f"w{kt}", tag=f"w{kt}") for kt in range(CT)]
            wp = [wpool.tile([128, C], FP16, name=f"wp{kt}", tag=f"wp{kt}") for kt in range(CT)]
            xf = [qkpool.tile([128, N], FP16, name=f"x{kt}", tag=f"x{kt}") for kt in range(CT)]
            xqt = [qkpool.tile([128, NQ], FP16, name=f"xq{kt}", tag=f"xq{kt}") for kt in range(CT)]

            def rows(kt):
                return slice(128 * kt, 128 * (kt + 1))

            # DMAs in first-use order. The 12 critical first pieces are
            # spread across the three HWDGE queues (sync/vector/scalar) so
            # their ~650ns per-DMA issue costs parallelize; bulk pieces go
            # through GPSIMD's software DGE.
            for kt in range(CT):  # first key columns via fast-issue SWDGE
                nc.gpsimd.dma_start(xf[kt][:, 0:1024], xk.ap()[rows(kt), 0:1024])
            for kt in range(CT):  # wq (all pairs) + wk(pair0) in one piece
                nc.sync.dma_start(wt[kt][:, 0:640], wqkv.ap()[rows(kt), 0:640])
            for kt in range(CT):  # query columns
                nc.sync.dma_start(xqt[kt][:, 0:512], xq.ap()[rows(kt), 0:512])
            for kt in range(CT):  # wk(pairs 1-3) + wv
                nc.gpsimd.dma_start(
                    wt[kt][:, 640 : 3 * C], wqkv.ap()[rows(kt), 640 : 3 * C]
                )
            for kt in range(CT):
                nc.gpsimd.dma_start(xf[kt][:, 1024:N], xk.ap()[rows(kt), 1024:N])
            for kt in range(CT):
                nc.gpsimd.dma_start(xqt[kt][:, 512:NQ], xq.ap()[rows(kt), 512:NQ])
            for kt in range(CT):
                nc.gpsimd.dma_start(wp[kt][:], wproj.ap()[rows(kt), :])

            qsb = [keep.tile([128, NQ], FP16, name=f"q{t}", tag=f"q{t}") for t in range(CT)]
            ksb = [keep.tile([128, N], FP16, name=f"k{t}", tag=f"k{t}") for t in range(CT)]
            vsb = [keep.tile([128, 520], FP16, name=f"v{m}", tag=f"v{m}") for m in range(MT)]
            avsb = [keep.tile([128, NQ], FP16, name=f"av{t}", tag=f"av{t}") for t in range(CT)]
            oa = [keep.tile([128, NQ], FP16, name=f"oa{t}", tag=f"oa{t}") for t in range(CT)]
            oy = [keep.tile([128, NQ], FP32, name=f"oy{t}", tag=f"oy{t}") for t in range(CT)]
            rec = [keep.tile([33, NQ], FP16, name=f"rc{t}", tag=f"rc{t}") for t in range(CT)]

            for t in range(CT):
                nc.gpsimd.memset(rec[t][0:32, :], 1.0)

            def qchunk(t, c0, cl):
                ps = ps1.tile([128, 512], FP32, name="ps1", tag="ps1")
                for kt in range(CT):
                    nc.tensor.matmul(
                        ps[:, 0:cl],
                        lhsT=wt[kt][:, 128 * t : 128 * (t + 1)],
                        rhs=xqt[kt][:, c0 : c0 + cl],
                        start=(kt == 0),
                        stop=(kt == CT - 1),
                    )
                nc.vector.tensor_copy(qsb[t][:, c0 : c0 + cl], ps[:, 0:cl])

            def kchunk(t, n0, nl):
                ps = ps1.tile([128, 512], FP32, name="ps1", tag="ps1")
                for kt in range(CT):
                    nc.tensor.matmul(
                        ps[:, 0:nl],
                        lhsT=wt[kt][:, C + 128 * t : C + 128 * (t + 1)],
                        rhs=xf[kt][:, n0 : n0 + nl],
                        start=(kt == 0),
                        stop=(kt == CT - 1),
                    )
                nc.vector.tensor_copy(ksb[t][:, n0 : n0 + nl], ps[:, 0:nl])

            def vblock(m):
                v3 = vsb[m][:].rearrange("p (h w) -> p h w", h=8)
                nc.vector.memset(v3[:, :, 64:65], 1.0)
                ps = ps1.tile([128, 512], FP32, name="ps1", tag="ps1")
                for kt in range(CT):
                    nc.tensor.matmul(
                        ps[:],
                        lhsT=xf[kt][:, 128 * m : 128 * (m + 1)],
                        rhs=wt[kt][:, 2 * C : 3 * C],
                        start=(kt == 0),
                        stop=(kt == CT - 1),
                    )
                nc.vector.tensor_copy(
                    v3[:, :, 0:64], ps[:].rearrange("p (h w) -> p h w", h=8)
                )

            def norm_chunk(t, c0, cl):
                bc = ps1.tile([128, 512], FP32, name="bc", tag="ps1")
                nc.tensor.matmul(
                    bc[:, 0:cl], lhsT=m2[:], rhs=rec[t][:, c0 : c0 + cl],
                    start=True, stop=True,
                )
                nc.vector.tensor_mul(
                    oa[t][:, c0 : c0 + cl], avsb[t][:, c0 : c0 + cl], bc[:, 0:cl]
                )

            def proj_chunk(c0, cl):
                for ct in range(CT):
                    py = ps1.tile([128, 512], FP32, name="py", tag="ps1")
                    for t in range(CT):
                        nc.tensor.matmul(
                            py[:, 0:cl],
                            lhsT=wp[t][:, 128 * ct : 128 * (ct + 1)],
                            rhs=oa[t][:, c0 : c0 + cl],
                            start=(t == 0),
                            stop=(t == CT - 1),
                        )
                    nc.vector.tensor_copy(oy[ct][:, c0 : c0 + cl], py[:, 0:cl])
                    eng = nc.scalar if (cl == 128 and ct % 2) else nc.sync
                    eng.dma_start(
                        y.ap()[128 * ct : 128 * (ct + 1), c0 : c0 + cl],
                        oy[ct][:, c0 : c0 + cl],
                    )

            def main_chunk(t, c0, cl, fill):
                """S+exp+AV m-loop for one (pair, query-chunk); fill is a
                list of emit-callbacks spread one per m-iteration."""
                kA = ksb[t][0:64, :]
                kB = ksb[t][64:128, :]
                qA = qsb[t][0:64, :]
                qB = qsb[t][64:128, :]
                avA = avp.tile([65, 512], FP32, name="avA", tag="avA")
                avB = avp.tile([65, 512], FP32, name="avB", tag="avB")
                pend = None
                pend2 = None

                def mk_av(m, es):
                    def emit():
                        nc.tensor.matmul(
                            avA[:], lhsT=vsb[m][:, 130 * t : 130 * t + 65],
                            rhs=es[:, 0:cl],
                            start=(m == 0), stop=(m == MT - 1),
                        )
                        nc.tensor.matmul(
                            avB[:], lhsT=vsb[m][:, 130 * t + 65 : 130 * t + 130],
                            rhs=es[:, 512 : 512 + cl],
                            start=(m == 0), stop=(m == MT - 1),
                        )
                    return emit

                for m in range(MT):
                    ms = slice(128 * m, 128 * (m + 1))
                    sp = spool.tile([128, 1024], FP32, name="s", tag="s")
                    nc.tensor.matmul(
                        sp[:, 0:cl], lhsT=kA[:, ms], rhs=qA[:, c0 : c0 + cl],
                        start=True, stop=True, tile_position=(0, 0),
                    )
                    nc.tensor.matmul(
                        sp[:, 512 : 512 + cl], lhsT=kB[:, ms], rhs=qB[:, c0 : c0 + cl],
                        start=True, stop=True, tile_position=(64, 0),
                    )
                    es = epool.tile([128, 1024], FP16, name="es", tag="es")
                    nc.scalar.activation(es[:], sp[:], AF.Exp, scale=SCALE)
                    if pend2 is not None:
                        pend2()
                    pend2 = pend
                    pend = mk_av(m, es)
                    if m < len(fill) and fill[m] is not None:
                        fill[m]()
                if pend2 is not None:
                    pend2()
                pend()
                nc.any.tensor_copy(avsb[t][0:64, c0 : c0 + cl], avA[0:64, :cl])
                nc.any.tensor_copy(avsb[t][64:128, c0 : c0 + cl], avB[0:64, :cl])
                with nc.allow_low_precision(reason="softmax recip fp16"):
                    nc.vector.reciprocal(rec[t][0:1, c0 : c0 + cl], avA[64:65, :cl])
                    nc.vector.reciprocal(rec[t][32:33, c0 : c0 + cl], avB[64:65, :cl])
                return lambda: norm_chunk(t, c0, cl)

            def rem_chunk(t, fill):
                """128-query remainder; exp batched over 4 key tiles."""
                kA = ksb[t][0:64, :]
                kB = ksb[t][64:128, :]
                qA = qsb[t][0:64, :]
                qB = qsb[t][64:128, :]
                avA = avp.tile([65, 128], FP32, name="avAr", tag="avA")
                avB = avp.tile([65, 128], FP32, name="avBr", tag="avB")
                pend = None
                pend2 = None

                def mk_av(g0, gm, es):
                    def emit():
                        for j in range(gm):
                            m = g0 + j
                            nc.tensor.matmul(
                                avA[:], lhsT=vsb[m][:, 130 * t : 130 * t + 65],
                                rhs=es[:, 128 * j : 128 * (j + 1)],
                                start=(m == 0), stop=(m == MT - 1),
                            )
                            nc.tensor.matmul(
                                avB[:], lhsT=vsb[m][:, 130 * t + 65 : 130 * t + 130],
                                rhs=es[:, 512 + 128 * j : 512 + 128 * (j + 1)],
                                start=(m == 0), stop=(m == MT - 1),
                            )
                    return emit

                gi = 0
                for g0 in range(0, MT, 4):
                    gm = min(4, MT - g0)
                    sp = spool.tile([128, 1024], FP32, name="sr", tag="s")
                    for j in range(gm):
                        ms = slice(128 * (g0 + j), 128 * (g0 + j + 1))
                        nc.tensor.matmul(
                            sp[:, 128 * j : 128 * (j + 1)], lhsT=kA[:, ms],
                            rhs=qA[:, 1024:1152],
                            start=True, stop=True, tile_position=(0, 0),
                        )
                        nc.tensor.matmul(
                            sp[:, 512 + 128 * j : 512 + 128 * (j + 1)], lhsT=kB[:, ms],
                            rhs=qB[:, 1024:1152],
                            start=True, stop=True, tile_position=(64, 0),
                        )
                    es = epool.tile([128, 1024], FP16, name="er", tag="es")
                    if gm == 4:
                        nc.scalar.activation(es[:], sp[:], AF.Exp, scale=SCALE)
                    else:
                        nc.scalar.activation(
                            es[:, 0 : 128 * gm], sp[:, 0 : 128 * gm], AF.Exp, scale=SCALE
                        )
                        nc.scalar.activation(
                            es[:, 512 : 512 + 128 * gm], sp[:, 512 : 512 + 128 * gm],
                            AF.Exp, scale=SCALE,
                        )
                    if pend2 is not None:
                        pend2()
                    pend2 = pend
                    pend = mk_av(g0, gm, es)
                    if gi < len(fill):
                        fill[gi]()
                        gi += 1
                if pend2 is not None:
                    pend2()
                pend()
                nc.any.tensor_copy(avsb[t][0:64, 1024:1152], avA[0:64, :])
                nc.any.tensor_copy(avsb[t][64:128, 1024:1152], avB[0:64, :])
                with nc.allow_low_precision(reason="softmax recip fp16"):
                    nc.vector.reciprocal(rec[t][0:1, 1024:1152], avA[64:65, :])
                    nc.vector.reciprocal(rec[t][32:33, 1024:1152], avB[64:65, :])
                return lambda: norm_chunk(t, 1024, 128)

            # ---- phase 1a: minimum prelude ----
            kchunk(0, 0, 128)
            qchunk(0, 0, 512)
            kchunk(0, 128, 384)
            for m in range(4):
                vblock(m)

            # ---- fillers (deadline-ordered) ----
            # pair-0 c0: rest of v (v[m] due by iteration m+2) + k chunks
            f_t0_c0 = []
            vq = list(range(4, MT))
            kq = [(0, n0, nl) for n0, nl in KCH[1:]]
            for i in range(MT):
                if i in (2, 6, 10, 14) and kq:
                    _, n0, nl = kq.pop(0)
                    f_t0_c0.append(lambda n0=n0, nl=nl: kchunk(0, n0, nl))
                elif vq:
                    m = vq.pop(0)
                    f_t0_c0.append(lambda m=m: vblock(m))
            while vq:
                m = vq.pop(0)
                f_t0_c0.append(lambda m=m: vblock(m))
            # q(0, c1-chunk) rides slot 8 as a second filler
            prev8 = f_t0_c0[8]

            def slot8(a=prev8):
                a()
                qchunk(0, 512, 512)
            f_t0_c0[8] = slot8

            def qk_fillers(t, q_first):
                out = []
                if q_first:
                    out.append(lambda: qchunk(t, 0, 512))
                for n0, nl in KCH:
                    out.append(lambda n0=n0, nl=nl: kchunk(t, n0, nl))
                if not q_first:
                    out.append(lambda: qchunk(t, 0, 512))
                out.append(lambda: qchunk(t, 512, 512))
                out.append(lambda: qchunk(t, 1024, 128))
                return out

            def spread(items, n=MT):
                """Place callbacks evenly across n slots (len(items) <= n)."""
                assert len(items) <= n
                out = [None] * n
                if not items:
                    return out
                step = n / len(items)
                for j, cb in enumerate(items):
                    pos = min(int(j * step), n - 1)
                    while out[pos] is not None:
                        pos += 1
                    out[pos] = cb
                return out

            # pair-0 c1 fillers: q(0, 512-chunk) is consumed by c1 itself and
            # is computed up-front below; remaining q(0) + all of pair 1.
            f_t0_c1 = [lambda: qchunk(0, 1024, 128)] + qk_fillers(1, True)

            fillers = {
                (0, 0): f_t0_c0,
                (0, 1): spread(f_t0_c1),
                (1, 0): spread(qk_fillers(2, True)[:5]),
                (1, 1): spread(qk_fillers(2, True)[5:]),
                (2, 0): spread(qk_fillers(3, True)[:5]),
                (2, 1): spread(qk_fillers(3, True)[5:]),
                (3, 0): [],
                (3, 1): [],
            }

            pending_norm = None

            def with_norm(fill):
                nonlocal_list = list(fill) if fill else [None] * MT
                while len(nonlocal_list) < MT:
                    nonlocal_list.append(None)
                return nonlocal_list

            def weave(fill, extra):
                out = with_norm(fill)
                if extra is None:
                    return out
                for i in range(MT):
                    if out[i] is None:
                        out[i] = extra
                        return out
                prev = out[0]

                def both(a=prev, b=extra):
                    if a is not None:
                        a()
                    b()
                out[0] = both
                return out

            for t in range(CT):
                if t == CT - 1:
                    fa = fillers[(t, 0)]
                    fb = fillers[(t, 1)]
                    pn = main_chunk(t, 0, 512, weave(fa, pending_norm))
                    pn2 = main_chunk(t, 512, 512, weave(fb, pn))
                    proj_chunk(0, 512)
                    pn3 = rem_chunk(t, [pn2])
                    proj_chunk(512, 512)
                    pn3()
                    proj_chunk(1024, 128)
                else:
                    fa = fillers[(t, 0)]
                    fb = fillers[(t, 1)]
                    pn = main_chunk(t, 0, 512, weave(fa, pending_norm))
                    pn2 = main_chunk(t, 512, 512, weave(fb, pn))
                    pending_norm = rem_chunk(t, [lambda: pn2()])

    nc.compile()
    return nc


def _get_module():
    if "nc" not in _CACHE:
        _CACHE["nc"] = _build_module()
    return _CACHE["nc"]


def make_in_maps(x, qkv_w, proj_w):
    xf = np.asarray(x, dtype=np.float32).reshape(B, C, N)
    wq = np.ascontiguousarray(np.asarray(qkv_w).T).astype(np.float16)
    wpj = np.ascontiguousarray(np.asarray(proj_w).T).astype(np.float16)
    in_maps = []
    for i in range(NCORES):
        b, h = divmod(i, 2)
        xkc = np.ascontiguousarray(xf[b]).astype(np.float16)
        xqc = np.ascontiguousarray(xf[b][:, h * NQ : (h + 1) * NQ]).astype(np.float16)
        in_maps.append({"xk": xkc, "xq": xqc, "wqkv": wq, "wproj": wpj})
    return in_maps


def gather_out(results):
    out = np.empty((B, C, N), np.float32)
    for i in range(NCORES):
        b, h = divmod(i, 2)
        out[b][:, h * NQ : (h + 1) * NQ] = results[i]["y"]
    return out.reshape(B, C, HH, WW)


def kernel(x, qkv_w, proj_w):
    from concourse import bass_utils

    nc = _get_module()
    in_maps = make_in_maps(x, qkv_w, proj_w)
    res = bass_utils.run_bass_kernel_spmd(
        nc, in_maps, core_ids=list(range(NCORES)), trace=False
    )
    return gather_out(res.results)


# revision 3
# speedup vs baseline: 1.0047x; 1.0047x over previous
"""Multi-head self-attention 2D Bass kernel for Trainium2.

Problem: x [4, 512, 48, 48] fp32; qkv_w [1536, 512]; proj_w [512, 512].
  qkv 1x1-conv -> per-head attention (8 heads, head_dim 64) over N=2304
  spatial positions -> output projection.

Sharding (8 cores): core i handles batch b = i//2 and query half i%2
  (NQ = 1152 queries). Each core computes K/V for the full image and the
  projection for its query columns; per-core outputs are disjoint slices
  of the full output -> host gather, no collectives.

Per-core pipeline. ScalarE (exp, ~171 us busy) and TensorE (~186 us
busy) are both near-saturated; the kernel is organized so neither waits:

  - Scores for a head pair are computed transposed (S^T = k_h^T q_h) as
    row-packed PE pairs (tile_position (0,0)/(64,0)); one wide exp per
    key-tile on ScalarE (softmax scale folded into the activation; max-
    subtraction skipped since scores*scale ~ N(0,1)); AV matmuls carry a
    ones column in the stationary operand so softmax denominators fall
    out of the same accumulation (PSUM row 64).
  - Software pipelining: AV matmuls trail the score/exp stream by two
    key-tiles, so the PE queue always prefers the matmuls that feed
    ScalarE next.
  - qkv phase interleaves with attention: only k/q for pair 0 plus
    v0..v3 run up-front; everything else (v4..v17, later pairs' k/q) is
    emitted as deadline-ordered fillers inside the attention m-loops,
    soaking up the PE slack in ScalarE-bound stretches.
  - Input DMAs are split into exactly the ranges the first blocks need
    and ordered by first use; bulk pieces ride GPSIMD's software DGE so
    the per-DMA HWDGE overhead stays off the critical path.
  - Normalization per (pair, chunk): reciprocal of the denominator rows
    straight from PSUM into rows {0,32} of a [33, NQ] tile (1-row DVE
    accesses must be 32-aligned), one [33,128] block-matrix matmul
    broadcasts both rows to 128 partitions, one tensor_mul applies it.
    The whole group is deferred into the next chunk's loop so chunk
    boundaries never stall either engine.
  - Output projection accumulates over head pairs in PSUM per query
    chunk as soon as the last pair normalizes that chunk; pair 3
    processes the 128-query remainder last so the kernel tail is only
    that small chunk's drain; output DMA per (row-tile, chunk).
"""

import numpy as np

B = 4
C = 512
HH = 48
WW = 48
N = HH * WW          # 2304
NQ = N // 2          # 1152 queries per core
HEADS = 8
D = C // HEADS       # 64
SCALE = float(D) ** -0.5
NCORES = 8

CT = C // 128        # 4 channel tiles
MT = N // 128        # 18 key tiles
QCH = [(0, 512), (512, 512), (1024, 128)]           # query chunks
KCH = [(0, 512), (512, 512), (1024, 512), (1536, 512), (2048, 256)]

_CACHE: dict = {}


def _build_module():
    import concourse.mybir as mybir
    import concourse.tile as tile
    from concourse import bacc

    FP16 = mybir.dt.float16
    FP32 = mybir.dt.float32
    AF = mybir.ActivationFunctionType

    nc = bacc.Bacc("TRN2", target_bir_lowering=False, debug=False)
    xk = nc.dram_tensor("xk", [C, N], FP16, kind="ExternalInput")
    xq = nc.dram_tensor("xq", [C, NQ], FP16, kind="ExternalInput")
    wqkv = nc.dram_tensor("wqkv", [C, 3 * C], FP16, kind="ExternalInput")
    wproj = nc.dram_tensor("wproj", [C, C], FP16, kind="ExternalInput")
    y = nc.dram_tensor("y", [C, NQ], FP32, kind="ExternalOutput")

    with tile.TileContext(nc) as tc:
        with (
            tc.tile_pool(name="consts", bufs=1) as cpool,
            tc.tile_pool(name="wts", bufs=1) as wpool,
            tc.tile_pool(name="qkv", bufs=1) as qkpool,
            tc.tile_pool(name="keep", bufs=1) as keep,
            tc.tile_pool(name="esb", bufs=10) as epool,
            tc.tile_pool(name="ps1", bufs=2, space="PSUM") as ps1,
            tc.tile_pool(name="sps", bufs=2, space="PSUM") as spool,
            tc.tile_pool(name="avps", bufs=1, space="PSUM") as avp,
        ):
            # block-broadcast matrix: out rows 0:64 <- rhs row0, 64:128 <- row32
            m2 = cpool.tile([33, 128], FP16, name="m2", tag="m2")
            nc.vector.memset(m2[0:32, :], 0.0)
            nc.vector.memset(m2[32:33, :], 0.0)
            nc.vector.memset(m2[0:1, 0:64], 1.0)
            nc.vector.memset(m2[32:33, 64:128], 1.0)

            wt = [wpool.tile([128, 3 * C], FP16, name=f"w{kt}", tag=f"w{kt}") for kt in range(CT)]
            wp = [wpool.tile([128, C], FP16, name=f"wp{kt}", tag=f"wp{kt}") for kt in range(CT)]
            xf = [qkpool.tile([128, N], FP16, name=f"x{kt}", tag=f"x{kt}") for kt in range(CT)]
            xqt = [qkpool.tile([128, NQ], FP16, name=f"xq{kt}", tag=f"xq{kt}") for kt in range(CT)]

            def rows(kt):
                return slice(128 * kt, 128 * (kt + 1))

            # DMAs in first-use order. The 12 critical first pieces are
            # spread across the three HWDGE queues (sync/vector/scalar) so
            # their ~650ns per-DMA issue costs parallelize; bulk pieces go
            # through GPSIMD's software DGE.
            for kt in range(CT):  # first key columns via fast-issue SWDGE
                nc.gpsimd.dma_start(xf[kt][:, 0:1024], xk.ap()[rows(kt), 0:1024])
            for kt in range(CT):  # wq (all pairs) + wk(pair0) in one piece
                nc.sync.dma_start(wt[kt][:, 0:640], wqkv.ap()[rows(kt), 0:640])
            for kt in range(CT):  # query columns
                nc.sync.dma_start(xqt[kt][:, 0:512], xq.ap()[rows(kt), 0:512])
            for kt in range(CT):  # wk(pairs 1-3) + wv
                nc.gpsimd.dma_start(
                    wt[kt][:, 640 : 3 * C], wqkv.ap()[rows(kt), 640 : 3 * C]
                )
            for kt in range(CT):
                nc.gpsimd.dma_start(xf[kt][:, 1024:N], xk.ap()[rows(kt), 1024:N])
            for kt in range(CT):
                nc.gpsimd.dma_start(xqt[kt][:, 512:NQ], xq.ap()[rows(kt), 512:NQ])
            for kt in range(CT):
                nc.gpsimd.dma_start(wp[kt][:], wproj.ap()[rows(kt), :])

            qsb = [keep.tile([128, NQ], FP16, name=f"q{t}", tag=f"q{t}") for t in range(CT)]
            ksb = [keep.tile([128, N], FP16, name=f"k{t}", tag=f"k{t}") for t in range(CT)]
            vsb = [keep.tile([128, 520], FP16, name=f"v{m}", tag=f"v{m}") for m in range(MT)]
            avsb = [keep.tile([128, NQ], FP16, name=f"av{t}", tag=f"av{t}") for t in range(CT)]
            oa = [keep.tile([128, NQ], FP16, name=f"oa{t}", tag=f"oa{t}") for t in range(CT)]
            oy = [keep.tile([128, NQ], FP32, name=f"oy{t}", tag=f"oy{t}") for t in range(CT)]
            rec = [keep.tile([33, NQ], FP16, name=f"rc{t}", tag=f"rc{t}") for t in range(CT)]

            for t in range(CT):
                nc.gpsimd.memset(rec[t][0:32, :], 1.0)

            def qchunk(t, c0, cl):
                ps = ps1.tile([128, 512], FP32, name="ps1", tag="ps1")
                for kt in range(CT):
                    nc.tensor.matmul(
                        ps[:, 0:cl],
                        lhsT=wt[kt][:, 128 * t : 128 * (t + 1)],
                        rhs=xqt[kt][:, c0 : c0 + cl],
                        start=(kt == 0),
                        stop=(kt == CT - 1),
                    )
                nc.vector.tensor_copy(qsb[t][:, c0 : c0 + cl], ps[:, 0:cl])

            def kchunk(t, n0, nl):
                ps = ps1.tile([128, 512], FP32, name="ps1", tag="ps1")
                for kt in range(CT):
                    nc.tensor.matmul(
                        ps[:, 0:nl],
                        lhsT=wt[kt][:, C + 128 * t : C + 128 * (t + 1)],
                        rhs=xf[kt][:, n0 : n0 + nl],
                        start=(kt == 0),
                        stop=(kt == CT - 1),
                    )
                nc.vector.tensor_copy(ksb[t][:, n0 : n0 + nl], ps[:, 0:nl])

            def vblock(m):
                v3 = vsb[m][:].rearrange("p (h w) -> p h w", h=8)
                nc.vector.memset(v3[:, :, 64:65], 1.0)
                ps = ps1.tile([128, 512], FP32, name="ps1", tag="ps1")
                for kt in range(CT):
                    nc.tensor.matmul(
                        ps[:],
                        lhsT=xf[kt][:, 128 * m : 128 * (m + 1)],
                        rhs=wt[kt][:, 2 * C : 3 * C],
                        start=(kt == 0),
                        stop=(kt == CT - 1),
                    )
                nc.vector.tensor_copy(
                    v3[:, :, 0:64], ps[:].rearrange("p (h w) -> p h w", h=8)
                )

            def norm_chunk(t, c0, cl):
                bc = ps1.tile([128, 512], FP32, name="bc", tag="ps1")
                nc.tensor.matmul(
                    bc[:, 0:cl], lhsT=m2[:], rhs=rec[t][:, c0 : c0 + cl],
                    start=True, stop=True,
                )
                nc.vector.tensor_mul(
                    oa[t][:, c0 : c0 + cl], avsb[t][:, c0 : c0 + cl], bc[:, 0:cl]
                )

            def proj_chunk(c0, cl):
                for ct in range(CT):
                    py = ps1.tile([128, 512], FP32, name="py", tag="ps1")
                    for t in range(CT):
                        nc.tensor.matmul(
                            py[:, 0:cl],
                            lhsT=wp[t][:, 128 * ct : 128 * (ct + 1)],
                            rhs=oa[t][:, c0 : c0 + cl],
                            start=(t == 0),
                            stop=(t == CT - 1),
                        )
                    nc.vector.tensor_copy(oy[ct][:, c0 : c0 + cl], py[:, 0:cl])
                    eng = nc.scalar if (cl == 128 and ct % 2) else nc.sync
                    eng.dma_start(
                        y.ap()[128 * ct : 128 * (ct + 1), c0 : c0 + cl],
                        oy[ct][:, c0 : c0 + cl],
                    )

            def main_chunk(t, c0, cl, fill):
                """S+exp+AV m-loop for one (pair, query-chunk); fill is a
                list of emit-callbacks spread one per m-iteration."""
                kA = ksb[t][0:64, :]
                kB = ksb[t][64:128, :]
                qA = qsb[t][0:64, :]
                qB = qsb[t][64:128, :]
                avA = avp.tile([65, 512], FP32, name="avA", tag="avA")
                avB = avp.tile([65, 512], FP32, name="avB", tag="avB")
                pend = None
                pend2 = None

                def mk_av(m, es):
                    def emit():
                        nc.tensor.matmul(
                            avA[:], lhsT=vsb[m][:, 130 * t : 130 * t + 65],
                            rhs=es[:, 0:cl],
                            start=(m == 0), stop=(m == MT - 1),
                        )
                        nc.tensor.matmul(
                            avB[:], lhsT=vsb[m][:, 130 * t + 65 : 130 * t + 130],
                            rhs=es[:, 512 : 512 + cl],
                            start=(m == 0), stop=(m == MT - 1),
                        )
                    return emit

                for m in range(MT):
                    ms = slice(128 * m, 128 * (m + 1))
                    sp = spool.tile([128, 1024], FP32, name="s", tag="s")
                    nc.tensor.matmul(
                        sp[:, 0:cl], lhsT=kA[:, ms], rhs=qA[:, c0 : c0 + cl],
                        start=True, stop=True, tile_position=(0, 0),
                    )
                    nc.tensor.matmul(
                        sp[:, 512 : 512 + cl], lhsT=kB[:, ms], rhs=qB[:, c0 : c0 + cl],
                        start=True, stop=True, tile_position=(64, 0),
                    )
                    es = epool.tile([128, 1024], FP16, name="es", tag="es")
                    nc.scalar.activation(es[:], sp[:], AF.Exp, scale=SCALE)
                    if pend2 is not None:
                        pend2()
                    pend2 = pend
                    pend = mk_av(m, es)
                    if m < len(fill) and fill[m] is not None:
                        fill[m]()
                if pend2 is not None:
                    pend2()
                pend()
                nc.vector.tensor_copy(avsb[t][0:64, c0 : c0 + cl], avA[0:64, :cl])
                nc.vector.tensor_copy(avsb[t][64:128, c0 : c0 + cl], avB[0:64, :cl])
                with nc.allow_low_precision(reason="softmax recip fp16"):
                    nc.vector.reciprocal(rec[t][0:1, c0 : c0 + cl], avA[64:65, :cl])
                    nc.vector.reciprocal(rec[t][32:33, c0 : c0 + cl], avB[64:65, :cl])
                return lambda: norm_chunk(t, c0, cl)

            def rem_chunk(t, fill):
                """128-query remainder; exp batched over 4 key tiles."""
                kA = ksb[t][0:64, :]
                kB = ksb[t][64:128, :]
                qA = qsb[t][0:64, :]
                qB = qsb[t][64:128, :]
                avA = avp.tile([65, 128], FP32, name="avAr", tag="avA")
                avB = avp.tile([65, 128], FP32, name="avBr", tag="avB")
                pend = None
                pend2 = None

                def mk_av(g0, gm, es):
                    def emit():
                        for j in range(gm):
                            m = g0 + j
                            nc.tensor.matmul(
                                avA[:], lhsT=vsb[m][:, 130 * t : 130 * t + 65],
                                rhs=es[:, 128 * j : 128 * (j + 1)],
                                start=(m == 0), stop=(m == MT - 1),
                            )
                            nc.tensor.matmul(
                                avB[:], lhsT=vsb[m][:, 130 * t + 65 : 130 * t + 130],
                                rhs=es[:, 512 + 128 * j : 512 + 128 * (j + 1)],
                                start=(m == 0), stop=(m == MT - 1),
                            )
                    return emit

                gi = 0
                for g0 in range(0, MT, 4):
                    gm = min(4, MT - g0)
                    sp = spool.tile([128, 1024], FP32, name="sr", tag="s")
                    for j in range(gm):
                        ms = slice(128 * (g0 + j), 128 * (g0 + j + 1))
                        nc.tensor.matmul(
                            sp[:, 128 * j : 128 * (j + 1)], lhsT=kA[:, ms],
                            rhs=qA[:, 1024:1152],
                            start=True, stop=True, tile_position=(0, 0),
                        )
                        nc.tensor.matmul(
                            sp[:, 512 + 128 * j : 512 + 128 * (j + 1)], lhsT=kB[:, ms],
                            rhs=qB[:, 1024:1152],
                            start=True, stop=True, tile_position=(64, 0),
                        )
                    es = epool.tile([128, 1024], FP16, name="er", tag="es")
                    if gm == 4:
                        nc.scalar.activation(es[:], sp[:], AF.Exp, scale=SCALE)
                    else:
                        nc.scalar.activation(
                            es[:, 0 : 128 * gm], sp[:, 0 : 128 * gm], AF.Exp, scale=SCALE
                        )
                        nc.scalar.activation(
                            es[:, 512 : 512 + 128 * gm], sp[:, 512 : 512 + 128 * gm],
                            AF.Exp, scale=SCALE,
                        )
                    if pend2 is not None:
                        pend2()
                    pend2 = pend
                    pend = mk_av(g0, gm, es)
                    if gi < len(fill):
                        fill[gi]()
                        gi += 1
                if pend2 is not None:
                    pend2()
                pend()
                nc.vector.tensor_copy(avsb[t][0:64, 1024:1152], avA[0:64, :])
                nc.vector.tensor_copy(avsb[t][64:128, 1024:1152], avB[0:64, :])
                with nc.allow_low_precision(reason="softmax recip fp16"):
                    nc.vector.reciprocal(rec[t][0:1, 1024:1152], avA[64:65, :])
                    nc.vector.reciprocal(rec[t][32:33, 1024:1152], avB[64:65, :])
                return lambda: norm_chunk(t, 1024, 128)

            # ---- phase 1a: minimum prelude ----
            kchunk(0, 0, 128)
            qchunk(0, 0, 512)
            kchunk(0, 128, 384)
            for m in range(4):
                vblock(m)

            # ---- fillers (deadline-ordered) ----
            # pair-0 c0: rest of v (v[m] due by iteration m+2) + k chunks
            f_t0_c0 = []
            vq = list(range(4, MT))
            kq = [(0, n0, nl) for n0, nl in KCH[1:]]
            for i in range(MT):
                if i in (2, 6, 10, 14) and kq:
                    _, n0, nl = kq.pop(0)
                    f_t0_c0.append(lambda n0=n0, nl=nl: kchunk(0, n0, nl))
                elif vq:
                    m = vq.pop(0)
                    f_t0_c0.append(lambda m=m: vblock(m))
            while vq:
                m = vq.pop(0)
                f_t0_c0.append(lambda m=m: vblock(m))
            # q(0, c1-chunk) rides slot 8 as a second filler
            prev8 = f_t0_c0[8]

            def slot8(a=prev8):
                a()
                qchunk(0, 512, 512)
            f_t0_c0[8] = slot8

            def qk_fillers(t, q_first):
                out = []
                if q_first:
                    out.append(lambda: qchunk(t, 0, 512))
                for n0, nl in KCH:
                    out.append(lambda n0=n0, nl=nl: kchunk(t, n0, nl))
                if not q_first:
                    out.append(lambda: qchunk(t, 0, 512))
                out.append(lambda: qchunk(t, 512, 512))
                out.append(lambda: qchunk(t, 1024, 128))
                return out

            def spread(items, n=MT):
                """Place callbacks evenly across n slots (len(items) <= n)."""
                assert len(items) <= n
                out = [None] * n
                if not items:
                    return out
                step = n / len(items)
                for j, cb in enumerate(items):
                    pos = min(int(j * step), n - 1)
                    while out[pos] is not None:
                        pos += 1
                    out[pos] = cb
                return out

            # pair-0 c1 fillers: q(0, 512-chunk) is consumed by c1 itself and
            # is computed up-front below; remaining q(0) + all of pair 1.
            f_t0_c1 = [lambda: qchunk(0, 1024, 128)] + qk_fillers(1, True)

            fillers = {
                (0, 0): f_t0_c0,
                (0, 1): spread(f_t0_c1),
                (1, 0): spread(qk_fillers(2, True)[:5]),
                (1, 1): spread(qk_fillers(2, True)[5:]),
                (2, 0): spread(qk_fillers(3, True)[:5]),
                (2, 1): spread(qk_fillers(3, True)[5:]),
                (3, 0): [],
                (3, 1): [],
            }

            pending_norm = None

            def with_norm(fill):
                nonlocal_list = list(fill) if fill else [None] * MT
                while len(nonlocal_list) < MT:
                    nonlocal_list.append(None)
                return nonlocal_list

            def weave(fill, extra):
                out = with_norm(fill)
                if extra is None:
                    return out
                for i in range(MT):
                    if out[i] is None:
                        out[i] = extra
                        return out
                prev = out[0]

                def both(a=prev, b=extra):
                    if a is not None:
                        a()
                    b()
                out[0] = both
                return out

            for t in range(CT):
                if t == CT - 1:
                    fa = fillers[(t, 0)]
                    fb = fillers[(t, 1)]
                    pn = main_chunk(t, 0, 512, weave(fa, pending_norm))
                    pn2 = main_chunk(t, 512, 512, weave(fb, pn))
                    proj_chunk(0, 512)
                    pn3 = rem_chunk(t, [pn2])
                    proj_chunk(512, 512)
                    pn3()
                    proj_chunk(1024, 128)
                else:
                    fa = fillers[(t, 0)]
                    fb = fillers[(t, 1)]
                    pn = main_chunk(t, 0, 512, weave(fa, pending_norm))
                    pn2 = main_chunk(t, 512, 512, weave(fb, pn))
                    pending_norm = rem_chunk(t, [lambda: pn2()])

    nc.compile()
    return nc


def _get_module():
    if "nc" not in _CACHE:
        _CACHE["nc"] = _build_module()
    return _CACHE["nc"]


def make_in_maps(x, qkv_w, proj_w):
    xf = np.asarray(x, dtype=np.float32).reshape(B, C, N)
    wq = np.ascontiguousarray(np.asarray(qkv_w).T).astype(np.float16)
    wpj = np.ascontiguousarray(np.asarray(proj_w).T).astype(np.float16)
    in_maps = []
    for i in range(NCORES):
        b, h = divmod(i, 2)
        xkc = np.ascontiguousarray(xf[b]).astype(np.float16)
        xqc = np.ascontiguousarray(xf[b][:, h * NQ : (h + 1) * NQ]).astype(np.float16)
        in_maps.append({"xk": xkc, "xq": xqc, "wqkv": wq, "wproj": wpj})
    return in_maps


def gather_out(results):
    out = np.empty((B, C, N), np.float32)
    for i in range(NCORES):
        b, h = divmod(i, 2)
        out[b][:, h * NQ : (h + 1) * NQ] = results[i]["y"]
    return out.reshape(B, C, HH, WW)


def kernel(x, qkv_w, proj_w):
    from concourse import bass_utils

    nc = _get_module()
    in_maps = make_in_maps(x, qkv_w, proj_w)
    res = bass_utils.run_bass_kernel_spmd(
        nc, in_maps, core_ids=list(range(NCORES)), trace=False
    )
    return gather_out(res.results)


# revision 4
# speedup vs baseline: 1.0136x; 1.0088x over previous
"""Multi-head self-attention 2D Bass kernel for Trainium2.

Problem: x [4, 512, 48, 48] fp32; qkv_w [1536, 512]; proj_w [512, 512].
  qkv 1x1-conv -> per-head attention (8 heads, head_dim 64) over N=2304
  spatial positions -> output projection.

Sharding (8 cores): core i handles batch b = i//2 and query half i%2
  (NQ = 1152 queries). Each core computes K/V for the full image and the
  projection for its query columns; per-core outputs are disjoint slices
  of the full output -> host gather, no collectives.

Per-core pipeline. ScalarE (exp, ~171 us busy) and TensorE (~186 us
busy) are both near-saturated; the kernel is organized so neither waits:

  - Scores for a head pair are computed transposed (S^T = k_h^T q_h) as
    row-packed PE pairs (tile_position (0,0)/(64,0)); one wide exp per
    key-tile on ScalarE (softmax scale folded into the activation; max-
    subtraction skipped since scores*scale ~ N(0,1)); AV matmuls carry a
    ones column in the stationary operand so softmax denominators fall
    out of the same accumulation (PSUM row 64).
  - Software pipelining: AV matmuls trail the score/exp stream by two
    key-tiles, so the PE queue always prefers the matmuls that feed
    ScalarE next.
  - qkv phase interleaves with attention: only k/q for pair 0 plus
    v0..v3 run up-front; everything else (v4..v17, later pairs' k/q) is
    emitted as deadline-ordered fillers inside the attention m-loops,
    soaking up the PE slack in ScalarE-bound stretches.
  - Input DMAs are split into exactly the ranges the first blocks need
    and ordered by first use; bulk pieces ride GPSIMD's software DGE so
    the per-DMA HWDGE overhead stays off the critical path.
  - Normalization per (pair, chunk): reciprocal of the denominator rows
    straight from PSUM into rows {0,32} of a [33, NQ] tile (1-row DVE
    accesses must be 32-aligned), one [33,128] block-matrix matmul
    broadcasts both rows to 128 partitions, one tensor_mul applies it.
    The whole group is deferred into the next chunk's loop so chunk
    boundaries never stall either engine.
  - Output projection accumulates over head pairs in PSUM per query
    chunk as soon as the last pair normalizes that chunk; pair 3
    processes the 128-query remainder last so the kernel tail is only
    that small chunk's drain; output DMA per (row-tile, chunk).
"""

import numpy as np

B = 4
C = 512
HH = 48
WW = 48
N = HH * WW          # 2304
NQ = N // 2          # 1152 queries per core
HEADS = 8
D = C // HEADS       # 64
SCALE = float(D) ** -0.5
NCORES = 8

CT = C // 128        # 4 channel tiles
MT = N // 128        # 18 key tiles
QCH = [(0, 512), (512, 512), (1024, 128)]           # query chunks
KCH = [(0, 512), (512, 512), (1024, 512), (1536, 512), (2048, 256)]

_CACHE: dict = {}


def _build_module():
    import concourse.mybir as mybir
    import concourse.tile as tile
    from concourse import bacc

    FP16 = mybir.dt.float16
    FP32 = mybir.dt.float32
    AF = mybir.ActivationFunctionType

    nc = bacc.Bacc("TRN2", target_bir_lowering=False, debug=False)
    xk = nc.dram_tensor("xk", [C, N], FP16, kind="ExternalInput")
    xq = nc.dram_tensor("xq", [C, NQ], FP16, kind="ExternalInput")
    wqkv = nc.dram_tensor("wqkv", [C, 3 * C], FP16, kind="ExternalInput")
    wproj = nc.dram_tensor("wproj", [C, C], FP16, kind="ExternalInput")
    y = nc.dram_tensor("y", [C, NQ], FP32, kind="ExternalOutput")

    with tile.TileContext(nc) as tc:
        with (
            tc.tile_pool(name="consts", bufs=1) as cpool,
            tc.tile_pool(name="wts", bufs=1) as wpool,
            tc.tile_pool(name="qkv", bufs=1) as qkpool,
            tc.tile_pool(name="keep", bufs=1) as keep,
            tc.tile_pool(name="esb", bufs=10) as epool,
            tc.tile_pool(name="ps1", bufs=2, space="PSUM") as ps1,
            tc.tile_pool(name="sps", bufs=2, space="PSUM") as spool,
            tc.tile_pool(name="avps", bufs=1, space="PSUM") as avp,
        ):
            # block-broadcast matrix: out rows 0:64 <- rhs row0, 64:128 <- row32
            m2 = cpool.tile([33, 128], FP16, name="m2", tag="m2")
            nc.vector.memset(m2[0:32, :], 0.0)
            nc.vector.memset(m2[32:33, :], 0.0)
            nc.vector.memset(m2[0:1, 0:64], 1.0)
            nc.vector.memset(m2[32:33, 64:128], 1.0)

            wt = [wpool.tile([128, 3 * C], FP16, name=f"w{kt}", tag=f"w{kt}") for kt in range(CT)]
            wp = [wpool.tile([128, C], FP16, name=f"wp{kt}", tag=f"wp{kt}") for kt in range(CT)]
            xf = [qkpool.tile([128, N], FP16, name=f"x{kt}", tag=f"x{kt}") for kt in range(CT)]
            xqt = [qkpool.tile([128, NQ], FP16, name=f"xq{kt}", tag=f"xq{kt}") for kt in range(CT)]

            def rows(kt):
                return slice(128 * kt, 128 * (kt + 1))

            # DMAs in first-use order. The 12 critical first pieces are
            # spread across the three HWDGE queues (sync/vector/scalar) so
            # their ~650ns per-DMA issue costs parallelize; bulk pieces go
            # through GPSIMD's software DGE.
            for kt in range(CT):  # first key columns via fast-issue SWDGE
                nc.gpsimd.dma_start(xf[kt][:, 0:1024], xk.ap()[rows(kt), 0:1024])
            for kt in range(CT):  # wq (all pairs) + wk(pair0) in one piece
                nc.sync.dma_start(wt[kt][:, 0:640], wqkv.ap()[rows(kt), 0:640])
            for kt in range(CT):  # query columns
                nc.sync.dma_start(xqt[kt][:, 0:512], xq.ap()[rows(kt), 0:512])
            for kt in range(CT):  # wk(pairs 1-3) + wv
                nc.gpsimd.dma_start(
                    wt[kt][:, 640 : 3 * C], wqkv.ap()[rows(kt), 640 : 3 * C]
                )
            for kt in range(CT):
                nc.gpsimd.dma_start(xf[kt][:, 1024:N], xk.ap()[rows(kt), 1024:N])
            for kt in range(CT):
                nc.gpsimd.dma_start(xqt[kt][:, 512:NQ], xq.ap()[rows(kt), 512:NQ])
            for kt in range(CT):
                nc.gpsimd.dma_start(wp[kt][:], wproj.ap()[rows(kt), :])

            qsb = [keep.tile([128, NQ], FP16, name=f"q{t}", tag=f"q{t}") for t in range(CT)]
            ksb = [keep.tile([128, N], FP16, name=f"k{t}", tag=f"k{t}") for t in range(CT)]
            vsb = [keep.tile([128, 520], FP16, name=f"v{m}", tag=f"v{m}") for m in range(MT)]
            avsb = [keep.tile([128, NQ], FP16, name=f"av{t}", tag=f"av{t}") for t in range(CT)]
            oa = [keep.tile([128, NQ], FP16, name=f"oa{t}", tag=f"oa{t}") for t in range(CT)]
            oy = [keep.tile([128, NQ], FP32, name=f"oy{t}", tag=f"oy{t}") for t in range(CT)]
            rec = [keep.tile([33, NQ], FP16, name=f"rc{t}", tag=f"rc{t}") for t in range(CT)]

            for t in range(CT):
                nc.gpsimd.memset(rec[t][0:32, :], 1.0)

            def qchunk(t, c0, cl):
                ps = ps1.tile([128, 512], FP32, name="ps1", tag="ps1")
                for kt in range(CT):
                    nc.tensor.matmul(
                        ps[:, 0:cl],
                        lhsT=wt[kt][:, 128 * t : 128 * (t + 1)],
                        rhs=xqt[kt][:, c0 : c0 + cl],
                        start=(kt == 0),
                        stop=(kt == CT - 1),
                    )
                nc.vector.tensor_copy(qsb[t][:, c0 : c0 + cl], ps[:, 0:cl])

            def kchunk(t, n0, nl):
                ps = ps1.tile([128, 512], FP32, name="ps1", tag="ps1")
                for kt in range(CT):
                    nc.tensor.matmul(
                        ps[:, 0:nl],
                        lhsT=wt[kt][:, C + 128 * t : C + 128 * (t + 1)],
                        rhs=xf[kt][:, n0 : n0 + nl],
                        start=(kt == 0),
                        stop=(kt == CT - 1),
                    )
                nc.vector.tensor_copy(ksb[t][:, n0 : n0 + nl], ps[:, 0:nl])

            def vblock(m):
                v3 = vsb[m][:].rearrange("p (h w) -> p h w", h=8)
                nc.vector.memset(v3[:, :, 64:65], 1.0)
                ps = ps1.tile([128, 512], FP32, name="ps1", tag="ps1")
                for kt in range(CT):
                    nc.tensor.matmul(
                        ps[:],
                        lhsT=xf[kt][:, 128 * m : 128 * (m + 1)],
                        rhs=wt[kt][:, 2 * C : 3 * C],
                        start=(kt == 0),
                        stop=(kt == CT - 1),
                    )
                nc.vector.tensor_copy(
                    v3[:, :, 0:64], ps[:].rearrange("p (h w) -> p h w", h=8)
                )

            def norm_chunk(t, c0, cl):
                bc = ps1.tile([128, 512], FP32, name="bc", tag="ps1")
                nc.tensor.matmul(
                    bc[:, 0:cl], lhsT=m2[:], rhs=rec[t][:, c0 : c0 + cl],
                    start=True, stop=True,
                )
                nc.vector.tensor_mul(
                    oa[t][:, c0 : c0 + cl], avsb[t][:, c0 : c0 + cl], bc[:, 0:cl]
                )

            def proj_chunk(c0, cl):
                for ct in range(CT):
                    py = ps1.tile([128, 512], FP32, name="py", tag="ps1")
                    for t in range(CT):
                        nc.tensor.matmul(
                            py[:, 0:cl],
                            lhsT=wp[t][:, 128 * ct : 128 * (ct + 1)],
                            rhs=oa[t][:, c0 : c0 + cl],
                            start=(t == 0),
                            stop=(t == CT - 1),
                        )
                    nc.vector.tensor_copy(oy[ct][:, c0 : c0 + cl], py[:, 0:cl])
                    eng = nc.scalar if (cl == 128 and ct % 2) else nc.sync
                    eng.dma_start(
                        y.ap()[128 * ct : 128 * (ct + 1), c0 : c0 + cl],
                        oy[ct][:, c0 : c0 + cl],
                    )

            def main_chunk(t, c0, cl, fill):
                """S+exp+AV m-loop for one (pair, query-chunk); fill is a
                list of emit-callbacks spread one per m-iteration."""
                kA = ksb[t][0:64, :]
                kB = ksb[t][64:128, :]
                qA = qsb[t][0:64, :]
                qB = qsb[t][64:128, :]
                avA = avp.tile([65, 512], FP32, name="avA", tag="avA")
                avB = avp.tile([65, 512], FP32, name="avB", tag="avB")
                pend = None
                pend2 = None

                def mk_av(m, es):
                    def emit():
                        nc.tensor.matmul(
                            avA[:], lhsT=vsb[m][:, 130 * t : 130 * t + 65],
                            rhs=es[:, 0:cl],
                            start=(m == 0), stop=(m == MT - 1),
                        )
                        nc.tensor.matmul(
                            avB[:], lhsT=vsb[m][:, 130 * t + 65 : 130 * t + 130],
                            rhs=es[:, 512 : 512 + cl],
                            start=(m == 0), stop=(m == MT - 1),
                        )
                    return emit

                for m in range(MT):
                    ms = slice(128 * m, 128 * (m + 1))
                    sp = spool.tile([128, 1024], FP32, name="s", tag="s")
                    nc.tensor.matmul(
                        sp[:, 0:cl], lhsT=kA[:, ms], rhs=qA[:, c0 : c0 + cl],
                        start=True, stop=True, tile_position=(0, 0),
                    )
                    nc.tensor.matmul(
                        sp[:, 512 : 512 + cl], lhsT=kB[:, ms], rhs=qB[:, c0 : c0 + cl],
                        start=True, stop=True, tile_position=(64, 0),
                    )
                    es = epool.tile([128, 1024], FP16, name="es", tag="es")
                    nc.scalar.activation(es[:], sp[:], AF.Exp, scale=SCALE)
                    if pend2 is not None:
                        pend2()
                    pend2 = pend
                    pend = mk_av(m, es)
                    if m < len(fill) and fill[m] is not None:
                        fill[m]()
                if pend2 is not None:
                    pend2()
                pend()
                nc.vector.tensor_copy(avsb[t][0:64, c0 : c0 + cl], avA[0:64, :cl])
                nc.vector.tensor_copy(avsb[t][64:128, c0 : c0 + cl], avB[0:64, :cl])
                with nc.allow_low_precision(reason="softmax recip fp16"):
                    nc.vector.reciprocal(rec[t][0:1, c0 : c0 + cl], avA[64:65, :cl])
                    nc.vector.reciprocal(rec[t][32:33, c0 : c0 + cl], avB[64:65, :cl])
                return lambda: norm_chunk(t, c0, cl)

            def rem_chunk(t, fill):
                """128-query remainder; exp batched over 4 key tiles."""
                kA = ksb[t][0:64, :]
                kB = ksb[t][64:128, :]
                qA = qsb[t][0:64, :]
                qB = qsb[t][64:128, :]
                avA = avp.tile([65, 128], FP32, name="avAr", tag="avA")
                avB = avp.tile([65, 128], FP32, name="avBr", tag="avB")
                pend = None
                pend2 = None

                def mk_av(g0, gm, es):
                    def emit():
                        for j in range(gm):
                            m = g0 + j
                            nc.tensor.matmul(
                                avA[:], lhsT=vsb[m][:, 130 * t : 130 * t + 65],
                                rhs=es[:, 128 * j : 128 * (j + 1)],
                                start=(m == 0), stop=(m == MT - 1),
                            )
                            nc.tensor.matmul(
                                avB[:], lhsT=vsb[m][:, 130 * t + 65 : 130 * t + 130],
                                rhs=es[:, 512 + 128 * j : 512 + 128 * (j + 1)],
                                start=(m == 0), stop=(m == MT - 1),
                            )
                    return emit

                gi = 0
                for g0 in range(0, MT, 4):
                    gm = min(4, MT - g0)
                    sp = spool.tile([128, 1024], FP32, name="sr", tag="s")
                    for j in range(gm):
                        ms = slice(128 * (g0 + j), 128 * (g0 + j + 1))
                        nc.tensor.matmul(
                            sp[:, 128 * j : 128 * (j + 1)], lhsT=kA[:, ms],
                            rhs=qA[:, 1024:1152],
                            start=True, stop=True, tile_position=(0, 0),
                        )
                        nc.tensor.matmul(
                            sp[:, 512 + 128 * j : 512 + 128 * (j + 1)], lhsT=kB[:, ms],
                            rhs=qB[:, 1024:1152],
                            start=True, stop=True, tile_position=(64, 0),
                        )
                    es = epool.tile([128, 1024], FP16, name="er", tag="es")
                    if gm == 4:
                        nc.scalar.activation(es[:], sp[:], AF.Exp, scale=SCALE)
                    else:
                        nc.scalar.activation(
                            es[:, 0 : 128 * gm], sp[:, 0 : 128 * gm], AF.Exp, scale=SCALE
                        )
                        nc.scalar.activation(
                            es[:, 512 : 512 + 128 * gm], sp[:, 512 : 512 + 128 * gm],
                            AF.Exp, scale=SCALE,
                        )
                    if pend2 is not None:
                        pend2()
                    pend2 = pend
                    pend = mk_av(g0, gm, es)
                    if gi < len(fill):
                        fill[gi]()
                        gi += 1
                if pend2 is not None:
                    pend2()
                pend()
                if t == CT - 1:
                    # kernel tail: ScalarE is idle after the last exp
                    nc.scalar.copy(avsb[t][0:64, 1024:1152], avA[0:64, :])
                    nc.scalar.copy(avsb[t][64:128, 1024:1152], avB[0:64, :])
                else:
                    nc.vector.tensor_copy(avsb[t][0:64, 1024:1152], avA[0:64, :])
                    nc.vector.tensor_copy(avsb[t][64:128, 1024:1152], avB[0:64, :])
                with nc.allow_low_precision(reason="softmax recip fp16"):
                    nc.vector.reciprocal(rec[t][0:1, 1024:1152], avA[64:65, :])
                    nc.vector.reciprocal(rec[t][32:33, 1024:1152], avB[64:65, :])
                return lambda: norm_chunk(t, 1024, 128)

            # ---- phase 1a: minimum prelude ----
            kchunk(0, 0, 128)
            qchunk(0, 0, 512)
            kchunk(0, 128, 384)
            for m in range(4):
                vblock(m)

            # ---- fillers (deadline-ordered) ----
            # pair-0 c0: rest of v (v[m] due by iteration m+2) + k chunks
            f_t0_c0 = []
            vq = list(range(4, MT))
            kq = [(0, n0, nl) for n0, nl in KCH[1:]]
            for i in range(MT):
                if i in (2, 6, 10, 14) and kq:
                    _, n0, nl = kq.pop(0)
                    f_t0_c0.append(lambda n0=n0, nl=nl: kchunk(0, n0, nl))
                elif vq:
                    m = vq.pop(0)
                    f_t0_c0.append(lambda m=m: vblock(m))
            while vq:
                m = vq.pop(0)
                f_t0_c0.append(lambda m=m: vblock(m))
            # q(0, c1-chunk) rides slot 12 as a second filler
            prev12 = f_t0_c0[12]

            def slot12(a=prev12):
                a()
                qchunk(0, 512, 512)
            f_t0_c0[12] = slot12

            def qk_fillers(t, q_first):
                out = []
                if q_first:
                    out.append(lambda: qchunk(t, 0, 512))
                for n0, nl in KCH:
                    out.append(lambda n0=n0, nl=nl: kchunk(t, n0, nl))
                if not q_first:
                    out.append(lambda: qchunk(t, 0, 512))
                out.append(lambda: qchunk(t, 512, 512))
                out.append(lambda: qchunk(t, 1024, 128))
                return out

            def spread(items, n=MT):
                """Place callbacks evenly across n slots (len(items) <= n)."""
                assert len(items) <= n
                out = [None] * n
                if not items:
                    return out
                step = n / len(items)
                for j, cb in enumerate(items):
                    pos = min(int(j * step), n - 1)
                    while out[pos] is not None:
                        pos += 1
                    out[pos] = cb
                return out

            # pair-0 c1 fillers: q(0, 512-chunk) is consumed by c1 itself and
            # is computed up-front below; remaining q(0) + all of pair 1.
            f_t0_c1 = [lambda: qchunk(0, 1024, 128)] + qk_fillers(1, True)

            fillers = {
                (0, 0): f_t0_c0,
                (0, 1): spread(f_t0_c1),
                (1, 0): spread(qk_fillers(2, True)[:4]),
                (1, 1): spread(qk_fillers(2, True)[4:]),
                (2, 0): spread(qk_fillers(3, True)[:4]),
                (2, 1): spread(qk_fillers(3, True)[4:]),
                (3, 0): [],
                (3, 1): [],
            }

            pending_norm = None

            def with_norm(fill):
                nonlocal_list = list(fill) if fill else [None] * MT
                while len(nonlocal_list) < MT:
                    nonlocal_list.append(None)
                return nonlocal_list

            def weave(fill, extra):
                out = with_norm(fill)
                if extra is None:
                    return out
                for i in range(MT):
                    if out[i] is None:
                        out[i] = extra
                        return out
                prev = out[0]

                def both(a=prev, b=extra):
                    if a is not None:
                        a()
                    b()
                out[0] = both
                return out

            for t in range(CT):
                if t == CT - 1:
                    fa = fillers[(t, 0)]
                    fb = fillers[(t, 1)]
                    pn = main_chunk(t, 0, 512, weave(fa, pending_norm))
                    pn2 = main_chunk(t, 512, 512, weave(fb, pn))
                    proj_chunk(0, 512)
                    pn3 = rem_chunk(t, [pn2])
                    proj_chunk(512, 512)
                    pn3()
                    proj_chunk(1024, 128)
                else:
                    fa = fillers[(t, 0)]
                    fb = fillers[(t, 1)]
                    pn = main_chunk(t, 0, 512, weave(fa, pending_norm))
                    pn2 = main_chunk(t, 512, 512, weave(fb, pn))
                    pending_norm = rem_chunk(t, [lambda: pn2()])

    nc.compile()
    return nc


def _get_module():
    if "nc" not in _CACHE:
        _CACHE["nc"] = _build_module()
    return _CACHE["nc"]


def make_in_maps(x, qkv_w, proj_w):
    xf = np.asarray(x, dtype=np.float32).reshape(B, C, N)
    wq = np.ascontiguousarray(np.asarray(qkv_w).T).astype(np.float16)
    wpj = np.ascontiguousarray(np.asarray(proj_w).T).astype(np.float16)
    in_maps = []
    for i in range(NCORES):
        b, h = divmod(i, 2)
        xkc = np.ascontiguousarray(xf[b]).astype(np.float16)
        xqc = np.ascontiguousarray(xf[b][:, h * NQ : (h + 1) * NQ]).astype(np.float16)
        in_maps.append({"xk": xkc, "xq": xqc, "wqkv": wq, "wproj": wpj})
    return in_maps


def gather_out(results):
    out = np.empty((B, C, N), np.float32)
    for i in range(NCORES):
        b, h = divmod(i, 2)
        out[b][:, h * NQ : (h + 1) * NQ] = results[i]["y"]
    return out.reshape(B, C, HH, WW)


def kernel(x, qkv_w, proj_w):
    from concourse import bass_utils

    nc = _get_module()
    in_maps = make_in_maps(x, qkv_w, proj_w)
    res = bass_utils.run_bass_kernel_spmd(
        nc, in_maps, core_ids=list(range(NCORES)), trace=False
    )
    return gather_out(res.results)


# revision 5
# speedup vs baseline: 1.0219x; 1.0082x over previous
"""Multi-head self-attention 2D Bass kernel for Trainium2.

Problem: x [4, 512, 48, 48] fp32; qkv_w [1536, 512]; proj_w [512, 512].
  qkv 1x1-conv -> per-head attention (8 heads, head_dim 64) over N=2304
  spatial positions -> output projection.

Sharding (8 cores): core i handles batch b = i//2 and query half i%2
  (NQ = 1152 queries). Each core computes K/V for the full image and the
  projection for its query columns; per-core outputs are disjoint slices
  of the full output -> host gather, no collectives.

Per-core pipeline. ScalarE (exp, ~171 us busy) and TensorE (~186 us
busy) are both near-saturated; the kernel is organized so neither waits:

  - Scores for a head pair are computed transposed (S^T = k_h^T q_h) as
    row-packed PE pairs (tile_position (0,0)/(64,0)); one wide exp per
    key-tile on ScalarE (softmax scale folded into the activation; max-
    subtraction skipped since scores*scale ~ N(0,1)); AV matmuls carry a
    ones column in the stationary operand so softmax denominators fall
    out of the same accumulation (PSUM row 64).
  - Software pipelining: AV matmuls trail the score/exp stream by two
    key-tiles, so the PE queue always prefers the matmuls that feed
    ScalarE next.
  - qkv phase interleaves with attention: only k/q for pair 0 plus
    v0..v3 run up-front; everything else (v4..v17, later pairs' k/q) is
    emitted as deadline-ordered fillers inside the attention m-loops,
    soaking up the PE slack in ScalarE-bound stretches.
  - Input DMAs are split into exactly the ranges the first blocks need
    and ordered by first use; bulk pieces ride GPSIMD's software DGE so
    the per-DMA HWDGE overhead stays off the critical path.
  - Normalization per (pair, chunk): reciprocal of the denominator rows
    straight from PSUM into rows {0,32} of a [33, NQ] tile (1-row DVE
    accesses must be 32-aligned), one [33,128] block-matrix matmul
    broadcasts both rows to 128 partitions, one tensor_mul applies it.
    The whole group is deferred into the next chunk's loop so chunk
    boundaries never stall either engine.
  - Output projection accumulates over head pairs in PSUM per query
    chunk as soon as the last pair normalizes that chunk; pair 3
    processes the 128-query remainder last so the kernel tail is only
    that small chunk's drain; output DMA per (row-tile, chunk).
"""

import numpy as np

B = 4
C = 512
HH = 48
WW = 48
N = HH * WW          # 2304
NQ = N // 2          # 1152 queries per core
HEADS = 8
D = C // HEADS       # 64
SCALE = float(D) ** -0.5
NCORES = 8

CT = C // 128        # 4 channel tiles
MT = N // 128        # 18 key tiles
QCH = [(0, 512), (512, 512), (1024, 128)]           # query chunks
KCH = [(0, 512), (512, 512), (1024, 512), (1536, 512), (2048, 256)]

_CACHE: dict = {}


def _build_module():
    import concourse.mybir as mybir
    import concourse.tile as tile
    from concourse import bacc

    FP16 = mybir.dt.float16
    FP32 = mybir.dt.float32
    AF = mybir.ActivationFunctionType

    nc = bacc.Bacc("TRN2", target_bir_lowering=False, debug=False)
    xk = nc.dram_tensor("xk", [C, N], FP16, kind="ExternalInput")
    xq = nc.dram_tensor("xq", [C, NQ], FP16, kind="ExternalInput")
    wqkv = nc.dram_tensor("wqkv", [C, 3 * C], FP16, kind="ExternalInput")
    wproj = nc.dram_tensor("wproj", [C, C], FP16, kind="ExternalInput")
    y = nc.dram_tensor("y", [C, NQ], FP32, kind="ExternalOutput")

    with tile.TileContext(nc) as tc:
        with (
            tc.tile_pool(name="consts", bufs=1) as cpool,
            tc.tile_pool(name="wts", bufs=1) as wpool,
            tc.tile_pool(name="qkv", bufs=1) as qkpool,
            tc.tile_pool(name="keep", bufs=1) as keep,
            tc.tile_pool(name="esb", bufs=10) as epool,
            tc.tile_pool(name="ps1", bufs=2, space="PSUM") as ps1,
            tc.tile_pool(name="sps", bufs=2, space="PSUM") as spool,
            tc.tile_pool(name="avps", bufs=1, space="PSUM") as avp,
        ):
            # block-broadcast matrix: out rows 0:64 <- rhs row0, 64:128 <- row32
            m2 = cpool.tile([33, 128], FP16, name="m2", tag="m2")
            nc.vector.memset(m2[0:32, :], 0.0)
            nc.vector.memset(m2[32:33, :], 0.0)
            nc.vector.memset(m2[0:1, 0:64], 1.0)
            nc.vector.memset(m2[32:33, 64:128], 1.0)

            wt = [wpool.tile([128, 3 * C], FP16, name=f"w{kt}", tag=f"w{kt}") for kt in range(CT)]
            wp = [wpool.tile([128, C], FP16, name=f"wp{kt}", tag=f"wp{kt}") for kt in range(CT)]
            xf = [qkpool.tile([128, N], FP16, name=f"x{kt}", tag=f"x{kt}") for kt in range(CT)]
            xqt = [qkpool.tile([128, NQ], FP16, name=f"xq{kt}", tag=f"xq{kt}") for kt in range(CT)]

            def rows(kt):
                return slice(128 * kt, 128 * (kt + 1))

            # DMAs in first-use order. The 12 critical first pieces are
            # spread across the three HWDGE queues (sync/vector/scalar) so
            # their ~650ns per-DMA issue costs parallelize; bulk pieces go
            # through GPSIMD's software DGE.
            for kt in range(CT):  # first key columns via fast-issue SWDGE
                nc.gpsimd.dma_start(xf[kt][:, 0:1024], xk.ap()[rows(kt), 0:1024])
            for kt in range(CT):  # wq (all pairs) + wk(pair0) in one piece
                nc.sync.dma_start(wt[kt][:, 0:640], wqkv.ap()[rows(kt), 0:640])
            for kt in range(CT):  # query columns
                nc.sync.dma_start(xqt[kt][:, 0:512], xq.ap()[rows(kt), 0:512])
            for kt in range(CT):  # wk(pairs 1-3) + wv
                nc.gpsimd.dma_start(
                    wt[kt][:, 640 : 3 * C], wqkv.ap()[rows(kt), 640 : 3 * C]
                )
            for kt in range(CT):
                nc.gpsimd.dma_start(xf[kt][:, 1024:N], xk.ap()[rows(kt), 1024:N])
            for kt in range(CT):
                nc.gpsimd.dma_start(xqt[kt][:, 512:NQ], xq.ap()[rows(kt), 512:NQ])
            for kt in range(CT):
                nc.gpsimd.dma_start(wp[kt][:], wproj.ap()[rows(kt), :])

            qsb = [keep.tile([128, NQ], FP16, name=f"q{t}", tag=f"q{t}") for t in range(CT)]
            ksb = [keep.tile([128, N], FP16, name=f"k{t}", tag=f"k{t}") for t in range(CT)]
            vsb = [keep.tile([128, 520], FP16, name=f"v{m}", tag=f"v{m}") for m in range(MT)]
            avsb = [keep.tile([128, NQ], FP16, name=f"av{t}", tag=f"av{t}") for t in range(CT)]
            oa = [keep.tile([128, NQ], FP16, name=f"oa{t}", tag=f"oa{t}") for t in range(CT)]
            oy = [keep.tile([128, NQ], FP32, name=f"oy{t}", tag=f"oy{t}") for t in range(CT)]
            rec = [keep.tile([33, NQ], FP16, name=f"rc{t}", tag=f"rc{t}") for t in range(CT)]

            for t in range(CT):
                nc.gpsimd.memset(rec[t][0:32, :], 1.0)

            def qchunk(t, c0, cl):
                ps = ps1.tile([128, 512], FP32, name="ps1", tag="ps1")
                for kt in range(CT):
                    nc.tensor.matmul(
                        ps[:, 0:cl],
                        lhsT=wt[kt][:, 128 * t : 128 * (t + 1)],
                        rhs=xqt[kt][:, c0 : c0 + cl],
                        start=(kt == 0),
                        stop=(kt == CT - 1),
                    )
                nc.vector.tensor_copy(qsb[t][:, c0 : c0 + cl], ps[:, 0:cl])

            def kchunk(t, n0, nl):
                ps = ps1.tile([128, 512], FP32, name="ps1", tag="ps1")
                for kt in range(CT):
                    nc.tensor.matmul(
                        ps[:, 0:nl],
                        lhsT=wt[kt][:, C + 128 * t : C + 128 * (t + 1)],
                        rhs=xf[kt][:, n0 : n0 + nl],
                        start=(kt == 0),
                        stop=(kt == CT - 1),
                    )
                nc.vector.tensor_copy(ksb[t][:, n0 : n0 + nl], ps[:, 0:nl])

            def vblock(m):
                v3 = vsb[m][:].rearrange("p (h w) -> p h w", h=8)
                nc.vector.memset(v3[:, :, 64:65], 1.0)
                ps = ps1.tile([128, 512], FP32, name="ps1", tag="ps1")
                for kt in range(CT):
                    nc.tensor.matmul(
                        ps[:],
                        lhsT=xf[kt][:, 128 * m : 128 * (m + 1)],
                        rhs=wt[kt][:, 2 * C : 3 * C],
                        start=(kt == 0),
                        stop=(kt == CT - 1),
                    )
                nc.vector.tensor_copy(
                    v3[:, :, 0:64], ps[:].rearrange("p (h w) -> p h w", h=8)
                )

            def norm_chunk(t, c0, cl):
                bc = ps1.tile([128, 512], FP32, name="bc", tag="ps1")
                nc.tensor.matmul(
                    bc[:, 0:cl], lhsT=m2[:], rhs=rec[t][:, c0 : c0 + cl],
                    start=True, stop=True,
                )
                nc.vector.tensor_mul(
                    oa[t][:, c0 : c0 + cl], avsb[t][:, c0 : c0 + cl], bc[:, 0:cl]
                )

            def proj_chunk(c0, cl):
                for ct in range(CT):
                    py = ps1.tile([128, 512], FP32, name="py", tag="ps1")
                    for t in range(CT):
                        nc.tensor.matmul(
                            py[:, 0:cl],
                            lhsT=wp[t][:, 128 * ct : 128 * (ct + 1)],
                            rhs=oa[t][:, c0 : c0 + cl],
                            start=(t == 0),
                            stop=(t == CT - 1),
                        )
                    nc.vector.tensor_copy(oy[ct][:, c0 : c0 + cl], py[:, 0:cl])
                    eng = nc.scalar if (cl == 128 and ct % 2) else nc.sync
                    eng.dma_start(
                        y.ap()[128 * ct : 128 * (ct + 1), c0 : c0 + cl],
                        oy[ct][:, c0 : c0 + cl],
                    )

            def main_chunk(t, c0, cl, fill):
                """S+exp+AV m-loop for one (pair, query-chunk); fill is a
                list of emit-callbacks spread one per m-iteration."""
                kA = ksb[t][0:64, :]
                kB = ksb[t][64:128, :]
                qA = qsb[t][0:64, :]
                qB = qsb[t][64:128, :]
                avA = avp.tile([65, 512], FP32, name="avA", tag="avA")
                avB = avp.tile([65, 512], FP32, name="avB", tag="avB")
                pend = None
                pend2 = None

                def mk_av(m, es):
                    def emit():
                        nc.tensor.matmul(
                            avA[:], lhsT=vsb[m][:, 130 * t : 130 * t + 65],
                            rhs=es[:, 0:cl],
                            start=(m == 0), stop=(m == MT - 1),
                        )
                        nc.tensor.matmul(
                            avB[:], lhsT=vsb[m][:, 130 * t + 65 : 130 * t + 130],
                            rhs=es[:, 512 : 512 + cl],
                            start=(m == 0), stop=(m == MT - 1),
                        )
                    return emit

                for m in range(MT):
                    ms = slice(128 * m, 128 * (m + 1))
                    sp = spool.tile([128, 1024], FP32, name="s", tag="s")
                    nc.tensor.matmul(
                        sp[:, 0:cl], lhsT=kA[:, ms], rhs=qA[:, c0 : c0 + cl],
                        start=True, stop=True, tile_position=(0, 0),
                    )
                    nc.tensor.matmul(
                        sp[:, 512 : 512 + cl], lhsT=kB[:, ms], rhs=qB[:, c0 : c0 + cl],
                        start=True, stop=True, tile_position=(64, 0),
                    )
                    es = epool.tile([128, 1024], FP16, name="es", tag="es")
                    nc.scalar.activation(es[:], sp[:], AF.Exp, scale=SCALE)
                    if pend2 is not None:
                        pend2()
                    pend2 = pend
                    pend = mk_av(m, es)
                    if m < len(fill) and fill[m] is not None:
                        fill[m]()
                if pend2 is not None:
                    pend2()
                pend()
                nc.vector.tensor_copy(avsb[t][0:64, c0 : c0 + cl], avA[0:64, :cl])
                nc.vector.tensor_copy(avsb[t][64:128, c0 : c0 + cl], avB[0:64, :cl])
                with nc.allow_low_precision(reason="softmax recip fp16"):
                    nc.vector.reciprocal(rec[t][0:1, c0 : c0 + cl], avA[64:65, :cl])
                    nc.vector.reciprocal(rec[t][32:33, c0 : c0 + cl], avB[64:65, :cl])
                return lambda: norm_chunk(t, c0, cl)

            def rem_chunk(t, fill):
                """128-query remainder; exp batched over 4 key tiles."""
                kA = ksb[t][0:64, :]
                kB = ksb[t][64:128, :]
                qA = qsb[t][0:64, :]
                qB = qsb[t][64:128, :]
                avA = avp.tile([65, 128], FP32, name="avAr", tag="avA")
                avB = avp.tile([65, 128], FP32, name="avBr", tag="avB")
                pend = None
                pend2 = None

                def mk_av(g0, gm, es):
                    def emit():
                        for j in range(gm):
                            m = g0 + j
                            nc.tensor.matmul(
                                avA[:], lhsT=vsb[m][:, 130 * t : 130 * t + 65],
                                rhs=es[:, 128 * j : 128 * (j + 1)],
                                start=(m == 0), stop=(m == MT - 1),
                            )
                            nc.tensor.matmul(
                                avB[:], lhsT=vsb[m][:, 130 * t + 65 : 130 * t + 130],
                                rhs=es[:, 512 + 128 * j : 512 + 128 * (j + 1)],
                                start=(m == 0), stop=(m == MT - 1),
                            )
                    return emit

                gi = 0
                for g0 in range(0, MT, 4):
                    gm = min(4, MT - g0)
                    sp = spool.tile([128, 1024], FP32, name="sr", tag="s")
                    for j in range(gm):
                        ms = slice(128 * (g0 + j), 128 * (g0 + j + 1))
                        nc.tensor.matmul(
                            sp[:, 128 * j : 128 * (j + 1)], lhsT=kA[:, ms],
                            rhs=qA[:, 1024:1152],
                            start=True, stop=True, tile_position=(0, 0),
                        )
                        nc.tensor.matmul(
                            sp[:, 512 + 128 * j : 512 + 128 * (j + 1)], lhsT=kB[:, ms],
                            rhs=qB[:, 1024:1152],
                            start=True, stop=True, tile_position=(64, 0),
                        )
                    es = epool.tile([128, 1024], FP16, name="er", tag="es")
                    if gm == 4:
                        nc.scalar.activation(es[:], sp[:], AF.Exp, scale=SCALE)
                    else:
                        nc.scalar.activation(
                            es[:, 0 : 128 * gm], sp[:, 0 : 128 * gm], AF.Exp, scale=SCALE
                        )
                        nc.scalar.activation(
                            es[:, 512 : 512 + 128 * gm], sp[:, 512 : 512 + 128 * gm],
                            AF.Exp, scale=SCALE,
                        )
                    if pend2 is not None:
                        pend2()
                    pend2 = pend
                    pend = mk_av(g0, gm, es)
                    if gi < len(fill):
                        fill[gi]()
                        gi += 1
                if pend2 is not None:
                    pend2()
                pend()
                if t == CT - 1:
                    # kernel tail: ScalarE is idle after the last exp
                    nc.scalar.copy(avsb[t][0:64, 1024:1152], avA[0:64, :])
                    nc.scalar.copy(avsb[t][64:128, 1024:1152], avB[0:64, :])
                else:
                    nc.vector.tensor_copy(avsb[t][0:64, 1024:1152], avA[0:64, :])
                    nc.vector.tensor_copy(avsb[t][64:128, 1024:1152], avB[0:64, :])
                with nc.allow_low_precision(reason="softmax recip fp16"):
                    nc.vector.reciprocal(rec[t][0:1, 1024:1152], avA[64:65, :])
                    nc.vector.reciprocal(rec[t][32:33, 1024:1152], avB[64:65, :])
                return lambda: norm_chunk(t, 1024, 128)

            # ---- phase 1a: minimum prelude ----
            kchunk(0, 0, 128)
            qchunk(0, 0, 512)
            kchunk(0, 128, 384)
            for m in range(4):
                vblock(m)

            # ---- fillers (deadline-ordered) ----
            # pair-0 c0: rest of v (v[m] due by iteration m+2) + k chunks
            f_t0_c0 = []
            vq = list(range(4, MT))
            kq = [(0, n0, nl) for n0, nl in KCH[1:]]
            for i in range(MT):
                if i in (2, 6, 10, 14) and kq:
                    _, n0, nl = kq.pop(0)
                    f_t0_c0.append(lambda n0=n0, nl=nl: kchunk(0, n0, nl))
                elif vq:
                    m = vq.pop(0)
                    f_t0_c0.append(lambda m=m: vblock(m))
            while vq:
                m = vq.pop(0)
                f_t0_c0.append(lambda m=m: vblock(m))
            # q(0, c1-chunk) rides slot 12 as a second filler
            prev12 = f_t0_c0[12]

            def slot12(a=prev12):
                a()
                qchunk(0, 512, 512)
            f_t0_c0[12] = slot12

            def qk_fillers(t, q_first):
                out = []
                if q_first:
                    out.append(lambda: qchunk(t, 0, 512))
                for n0, nl in KCH:
                    out.append(lambda n0=n0, nl=nl: kchunk(t, n0, nl))
                if not q_first:
                    out.append(lambda: qchunk(t, 0, 512))
                out.append(lambda: qchunk(t, 512, 512))
                out.append(lambda: qchunk(t, 1024, 128))
                return out

            def spread(items, n=MT):
                """Place callbacks evenly across n slots (len(items) <= n)."""
                assert len(items) <= n
                out = [None] * n
                if not items:
                    return out
                step = n / len(items)
                for j, cb in enumerate(items):
                    pos = min(int(j * step), n - 1)
                    while out[pos] is not None:
                        pos += 1
                    out[pos] = cb
                return out

            # pair-0 c1 fillers: q(0, 512-chunk) is consumed by c1 itself and
            # is computed up-front below; remaining q(0) + all of pair 1.
            f_t0_c1 = [lambda: qchunk(0, 1024, 128)] + qk_fillers(1, True)

            fillers = {
                (0, 0): f_t0_c0,
                (0, 1): spread(f_t0_c1),
                (1, 0): spread(qk_fillers(2, True)[:4]),
                (1, 1): spread(qk_fillers(2, True)[4:]),
                (2, 0): spread(qk_fillers(3, True)[:4]),
                (2, 1): spread(qk_fillers(3, True)[4:]),
                (3, 0): [],
                (3, 1): [],
            }

            pending_norm = None

            def with_norm(fill):
                nonlocal_list = list(fill) if fill else [None] * MT
                while len(nonlocal_list) < MT:
                    nonlocal_list.append(None)
                return nonlocal_list

            def weave(fill, extra):
                out = with_norm(fill)
                if extra is None:
                    return out
                for i in list(range(8, MT)) + list(range(8)):
                    if out[i] is None:
                        out[i] = extra
                        return out
                prev = out[0]

                def both(a=prev, b=extra):
                    if a is not None:
                        a()
                    b()
                out[0] = both
                return out

            for t in range(CT):
                if t == CT - 1:
                    fa = fillers[(t, 0)]
                    fb = fillers[(t, 1)]
                    pn = main_chunk(t, 0, 512, weave(fa, pending_norm))
                    pn2 = main_chunk(t, 512, 512, weave(fb, pn))
                    proj_chunk(0, 512)
                    pn3 = rem_chunk(t, [pn2])
                    proj_chunk(512, 512)
                    pn3()
                    proj_chunk(1024, 128)
                else:
                    fa = fillers[(t, 0)]
                    fb = fillers[(t, 1)]
                    pn = main_chunk(t, 0, 512, weave(fa, pending_norm))
                    pn2 = main_chunk(t, 512, 512, weave(fb, pn))
                    pending_norm = rem_chunk(t, [lambda: pn2()])

    nc.compile()
    return nc


def _get_module():
    if "nc" not in _CACHE:
        _CACHE["nc"] = _build_module()
    return _CACHE["nc"]


def make_in_maps(x, qkv_w, proj_w):
    xf = np.asarray(x, dtype=np.float32).reshape(B, C, N)
    wq = np.ascontiguousarray(np.asarray(qkv_w).T).astype(np.float16)
    wpj = np.ascontiguousarray(np.asarray(proj_w).T).astype(np.float16)
    in_maps = []
    for i in range(NCORES):
        b, h = divmod(i, 2)
        xkc = np.ascontiguousarray(xf[b]).astype(np.float16)
        xqc = np.ascontiguousarray(xf[b][:, h * NQ : (h + 1) * NQ]).astype(np.float16)
        in_maps.append({"xk": xkc, "xq": xqc, "wqkv": wq, "wproj": wpj})
    return in_maps


def gather_out(results):
    out = np.empty((B, C, N), np.float32)
    for i in range(NCORES):
        b, h = divmod(i, 2)
        out[b][:, h * NQ : (h + 1) * NQ] = results[i]["y"]
    return out.reshape(B, C, HH, WW)


def kernel(x, qkv_w, proj_w):
    from concourse import bass_utils

    nc = _get_module()
    in_maps = make_in_maps(x, qkv_w, proj_w)
    res = bass_utils.run_bass_kernel_spmd(
        nc, in_maps, core_ids=list(range(NCORES)), trace=False
    )
    return gather_out(res.results)
